# revision 14
# baseline (speedup 1.0000x reference)
"""Multi-head attention (B=4, L=2048, C=1024, H=16, HD=64) on 8 NeuronCores.

Sharding: tensor-parallel over heads — 2 heads per core. Each core computes
its heads' QKV projection, attention, and a partial output projection over
its 128 ctx channels; the host sums the 8 partial outputs.

Per-core kernel layout notes:
  - All projections/attention keep "T" layouts (channels on partitions);
    the whole data path is bf16 except PSUM accumulation (fp32) and the
    softmax normalization chain.
  - Scores matmuls are K=64 per head, row-tiled onto the PE array with
    tile_position (0,0)/(64,0) so both heads' scores run CONCURRENTLY in
    disjoint row halves of the systolic array (q/k tiles carry head0 dims
    on partitions 0:64 and head1 on 64:128, matching the array row groups).
  - ctx matmuls are M=64 per head, col-tiled with tile_position (0,0)/(0,64)
    so both heads' ctx accumulate CONCURRENTLY into partition halves of one
    PSUM bank.
  - Softmax skips the max-subtraction (scores are ~N(0, 1/9): exp is safe).
    The denominator Z comes from a bf16 pairwise tree-reduction of the 16
    exp tiles (DVE) + one M=1 ones-matmul per head; 1/Z via DVE
    reciprocal_approx_fast (no ACT Exp<->Ln table reloads).
  - DMA descriptors are merged (one DMA for all qkv weights, one per
    512-token x chunk, one per outproj row tile) to keep the Sync queue
    short — descriptor issue is ~0.6us each.
"""

import numpy as np
import ml_dtypes

import concourse.bass as bass
import concourse.mybir as mybir
import concourse.tile as tile
from concourse import bacc
from concourse.bass_utils import run_bass_kernel_spmd

B, L, C, H, HD = 4, 2048, 1024, 16, 64
NCORES = 8
HPC = H // NCORES  # heads per core = 2
F32 = mybir.dt.float32
F32R = mybir.dt.float32r
BF16 = mybir.dt.bfloat16
EXP = mybir.ActivationFunctionType.Exp

LCHUNK = 512          # token chunk for moving operands
NLC = L // LCHUNK     # 4
NKT = L // 128        # 16 k tiles per sequence
NCT = C // 128        # 8 contraction tiles for the projections


def build_kernel():
    nc = bacc.Bacc("TRN2", target_bir_lowering=False, debug=False,
                   num_devices=NCORES)

    xT = nc.dram_tensor("xT", [B, C, L], BF16, kind="ExternalInput")
    # wqkv[ci, j] = [128 c, 128 f] tile; j in (0=q both heads, 1=k, 2=v)
    wqkv = nc.dram_tensor("wqkv", [NCT, 3, 128, 128], BF16, kind="ExternalInput")
    bqkv = nc.dram_tensor("bqkv", [3, 128, 1], F32, kind="ExternalInput")
    # wo2: [128 c(2 heads), 1024 o]
    wo2 = nc.dram_tensor("wo2", [128, C], BF16, kind="ExternalInput")
    onesb_d = nc.dram_tensor("onesb_d", [128, 8], BF16, kind="ExternalInput")
    identb_d = nc.dram_tensor("identb_d", [128, 128], BF16, kind="ExternalInput")
    out = nc.dram_tensor("out", [B * L, C], F32, kind="ExternalOutput")

    with tile.TileContext(nc) as tc:
        kernel_body(nc, tc, xT, wqkv, bqkv, wo2, onesb_d, identb_d, out)
    nc.compile()
    return nc


def kernel_body(nc, tc, xT, wqkv, bqkv, wo2, onesb_d, identb_d, out):
    from contextlib import ExitStack
    ctx = ExitStack()
    with ctx:
        consts = ctx.enter_context(tc.tile_pool(name="consts", bufs=1))
        xpool = ctx.enter_context(tc.tile_pool(name="xpool", bufs=4))
        qkvpool = ctx.enter_context(tc.tile_pool(name="qkvpool", bufs=2))
        vppool = ctx.enter_context(tc.tile_pool(name="vppool", bufs=24))
        epool = ctx.enter_context(tc.tile_pool(name="epool", bufs=18))
        zpool = ctx.enter_context(tc.tile_pool(name="zpool", bufs=16))
        cpool = ctx.enter_context(tc.tile_pool(name="cpool", bufs=2))
        spool = ctx.enter_context(tc.tile_pool(name="spool", bufs=2))
        opool = ctx.enter_context(tc.tile_pool(name="opool", bufs=3))
        # PSUM banks: s pairs 2x2 + cacc 1 + general 3 = 8
        spsum = ctx.enter_context(tc.tile_pool(name="spsum", bufs=2,
                                               space="PSUM"))
        cpsum = ctx.enter_context(tc.tile_pool(name="cpsum", bufs=1,
                                               space="PSUM"))
        gpsum = ctx.enter_context(tc.tile_pool(name="gpsum", bufs=3,
                                               space="PSUM"))

        # ---- constants (single merged weight DMA) ----
        w_all = consts.tile([128, NCT, 3, 128], BF16, tag="w_all")
        nc.sync.dma_start(out=w_all, in_=wqkv[:].rearrange("a b c d -> c a b d"))
        w_tiles = [[w_all[:, ci, j, :] for j in range(3)] for ci in range(NCT)]
        b_tiles = []
        for j in range(3):
            t = consts.tile([128, 1], F32, tag=f"b{j}")
            nc.sync.dma_start(out=t, in_=bqkv[j])
            b_tiles.append(t)
        wo_t = consts.tile([128, C], BF16, tag="wo_t")
        nc.sync.dma_start(out=wo_t, in_=wo2[:])
        onesb = consts.tile([128, 8], BF16, tag="onesb")
        nc.sync.dma_start(out=onesb, in_=onesb_d[:])
        identb = consts.tile([128, 128], BF16, tag="identb")
        nc.sync.dma_start(out=identb, in_=identb_d[:])

        # ---- phase helpers (emitted in software-pipelined order below) ----
        def emit_qkv_loads(b, lc):
            ls = bass.ts(lc, LCHUNK)
            xt = xpool.tile([128, NCT, LCHUNK], BF16, tag="xt", name="xt")
            nc.sync.dma_start(
                out=xt, in_=xT[b, :, ls].rearrange("(a p) l -> p a l", p=128))
            return xt

        def emit_qkv_block(b, lc, qkvT, xt):
            ls = bass.ts(lc, LCHUNK)
            for j in range(3):
                p = gpsum.tile([128, LCHUNK], F32, tag="gpb", name="p")
                for ci in range(NCT):
                    nc.tensor.matmul(p, w_tiles[ci][j], xt[:, ci, :],
                                     start=(ci == 0), stop=(ci == NCT - 1))
                # PSUM -> SBUF with per-partition bias add
                nc.vector.tensor_scalar_add(qkvT[j][:, ls], p, b_tiles[j][:])

        def emit_vplus(qkvT):
            # v -> token-major bf16 tiles [128 l, v_h0 | v_h1]
            vplus = []
            for t in range(NKT):
                tp = gpsum.tile([128, 128], BF16, tag="gpb", name="tp")
                nc.tensor.transpose(tp, qkvT[2][:, bass.ts(t, 128)], identb[:])
                vp = vppool.tile([128, 2 * HD], BF16, tag="vp", name="vp")
                nc.vector.tensor_copy(vp, tp)
                vplus.append(vp)
            return vplus

        def emit_attn_chunk(qc, qkvT, vplus):
            # both heads, one 512-wide q chunk; scores run as K=64 row-tiled
            # pairs (head h in array rows 64h:64h+64), ctx as M=64 col-tiled
            # pairs (head h in array cols 64h:64h+64); one exp per k-tile
            # covering both heads' PSUM banks.
            qs = bass.ts(qc, LCHUNK)
            qt, kt = qkvT[0], qkvT[1]
            evec = []
            for i in range(NKT):
                ks = bass.ts(i, 128)
                s = spsum.tile([128, 2 * LCHUNK], F32, tag="spb", name="s")
                nc.tensor.matmul(s[:, 0:LCHUNK], kt[0:HD, ks], qt[0:HD, qs],
                                 start=True, stop=True, tile_position=(0, 0))
                nc.tensor.matmul(s[:, LCHUNK:2 * LCHUNK], kt[HD:128, ks],
                                 qt[HD:128, qs],
                                 start=True, stop=True, tile_position=(64, 0))
                e = epool.tile([128, 2 * LCHUNK], BF16, tag="e", name="e")
                nc.scalar.activation(e, s, EXP, scale=0.125)
                evec.append(e)
            # pairwise tree-reduction of the 16 exp tiles -> zacc (for Z)
            lvl = evec
            while len(lvl) > 1:
                nxt_lvl = []
                for a, bb in zip(lvl[0::2], lvl[1::2]):
                    t = zpool.tile([128, 2 * LCHUNK], BF16, tag="zt", name="zt")
                    nc.vector.tensor_add(t, a, bb)
                    nxt_lvl.append(t)
                lvl = nxt_lvl
            zacc = lvl[0]
            # ctx accumulation, both heads col-tiled into one PSUM bank
            cacc = cpsum.tile([128, LCHUNK], F32, tag="cpb", name="cacc")
            for i in range(NKT):
                for h in range(HPC):
                    nc.tensor.matmul(
                        cacc[h * HD:(h + 1) * HD, :],
                        vplus[i][:, bass.ts(h, HD)],
                        evec[i][:, bass.ts(h, LCHUNK)],
                        start=(i == 0), stop=(i == NKT - 1),
                        tile_position=(0, h * HD))
            # Z = ones.T @ zacc (M=1 matmuls into a spare spsum rotation)
            zp = spsum.tile([1, 2 * LCHUNK], F32, tag="spb", name="zp")
            for h in range(HPC):
                nc.tensor.matmul(zp[0:1, bass.ts(h, LCHUNK)], onesb[:, 0:1],
                                 zacc[:, bass.ts(h, LCHUNK)],
                                 start=True, stop=True)
            # drain cacc/zp immediately (frees the PSUM banks); per-head csb
            # at base partition 0 (DVE tensor_tensor inputs must share base)
            pend = []
            for h in range(HPC):
                csb = spool.tile([HD, LCHUNK], F32, tag="csb", name="csb",
                                 bufs=5)
                nc.vector.tensor_copy(csb, cacc[h * HD:(h + 1) * HD, :])
                z0 = spool.tile([1, LCHUNK], F32, tag="z0", name="z0", bufs=5)
                nc.vector.tensor_copy(z0[0:1, :], zp[0:1, bass.ts(h, LCHUNK)])
                pend.append((h, qs, csb, z0))
            return pend

        def emit_normalize(pend, ctxT2):
            # broadcast Z from partition 0 (GpSimd broadcast only reads
            # physical partition 0), then one full-width DVE reciprocal
            # and multiply per head
            for (h, qs, csb, z0) in pend:
                zb = spool.tile([HD, LCHUNK], F32, tag="zb", name="zb", bufs=3)
                nc.gpsimd.partition_broadcast(zb[0:HD, :], z0[0:1, :])
                zs = spool.tile([HD, LCHUNK], F32, tag="zs", name="zs", bufs=3)
                nc.vector.reciprocal_approx_fast(zs, zb)
                nc.vector.tensor_mul(ctxT2[h * HD:h * HD + HD, qs],
                                     csb[0:HD, :], zs)

        def emit_outproj(b, ctxT2, trange):
            for t in trange:
                rows = bass.ds(b * L + t * 128, 128)
                ot = opool.tile([128, C], F32, tag="ot", name="ot")
                for oc in range(C // 512):
                    os_ = bass.ts(oc, 512)
                    o = gpsum.tile([128, 512], F32, tag="gpb", name="o")
                    nc.tensor.matmul(o, ctxT2[:, bass.ts(t, 128)],
                                     wo_t[:, os_], start=True, stop=True)
                    nc.vector.tensor_copy(ot[:, os_], o)
                nc.sync.dma_start(out=out[rows, :], in_=ot)

        # ---- software-pipelined emission ----
        # Interleave next batch's qkv blocks between attention chunks so the
        # PE's in-order queue always has dense, ready work behind any stall.
        def new_qkvT():
            qp = qkvpool.tile([128, L], BF16, tag="qp", name="qp")
            kc = qkvpool.tile([128, L], BF16, tag="kc", name="kc")
            vc = qkvpool.tile([128, L], BF16, tag="vc", name="vc")
            return [qp, kc, vc]

        qkvT = new_qkvT()
        for lc in range(NLC):
            xt = emit_qkv_loads(0, lc)
            emit_qkv_block(0, lc, qkvT, xt)
        vplus = emit_vplus(qkvT)
        for b in range(B):
            ctxT2 = cpool.tile([128, L], BF16, tag="ctxT2", name="ctxT2")
            nxt = new_qkvT() if b + 1 < B else None
            loads = {}
            if nxt is not None:
                loads[0] = emit_qkv_loads(b + 1, 0)
                loads[1] = emit_qkv_loads(b + 1, 1)
            for k in range(NLC):
                pend = emit_attn_chunk(k, qkvT, vplus)
                if nxt is not None:
                    if k + 2 < NLC:
                        loads[k + 2] = emit_qkv_loads(b + 1, k + 2)
                    emit_qkv_block(b + 1, k, nxt, loads.pop(k))
                emit_normalize(pend, ctxT2)
                if k == 1:
                    emit_outproj(b, ctxT2, range(0, NKT // 2))
            if nxt is not None:
                nxt_vplus = emit_vplus(nxt)
            emit_outproj(b, ctxT2, range(NKT // 2, NKT))
            if nxt is not None:
                qkvT, vplus = nxt, nxt_vplus


_NC_CACHE = None


def get_nc():
    global _NC_CACHE
    if _NC_CACHE is None:
        _NC_CACHE = build_kernel()
    return _NC_CACHE


def prepare_in_maps(x, W_qkv, b_qkv, W_out, b_out):
    x = np.asarray(x, np.float32)
    W_qkv = np.asarray(W_qkv, np.float32)
    b_qkv = np.asarray(b_qkv, np.float32)
    W_out = np.asarray(W_out, np.float32)
    b_out = np.asarray(b_out, np.float32)

    xT = np.ascontiguousarray(x.transpose(0, 2, 1))  # [B, C, L]

    in_maps = []
    for core in range(NCORES):
        h0 = HPC * core
        # per-head channel rows in W_qkv: q = h*192..+64, k = +64, v = +128
        qrows = [np.arange(h * 192, h * 192 + 64) for h in (h0, h0 + 1)]
        krows = [q + 64 for q in qrows]
        vrows = [q + 128 for q in qrows]
        fq = np.concatenate(qrows)
        fk = np.concatenate(krows)
        fv = np.concatenate(vrows)
        # wqkv tiles: [ci, j, 128 c, 128 f]
        wt = np.empty((NCT, 3, 128, 128), ml_dtypes.bfloat16)
        for j, rows in enumerate((fq, fk, fv)):
            wT = np.ascontiguousarray(W_qkv[rows].T)  # [1024 c, 128 f]
            wt[:, j] = wT.reshape(NCT, 128, 128)
        bq = np.stack([b_qkv[fq], b_qkv[fk], b_qkv[fv]])[..., None]  # [3,128,1]
        # wo2 = [128 c, 1024 o]: rows 0:64 h0 ctx channels, 64:128 h1
        wo2 = np.concatenate([
            np.ascontiguousarray(W_out[:, (h0 + h) * HD:(h0 + h + 1) * HD].T)
            for h in range(HPC)
        ], axis=0)
        in_maps.append({
            "xT": xT.astype(ml_dtypes.bfloat16),
            "wqkv": wt,
            "bqkv": np.ascontiguousarray(bq),
            "wo2": np.ascontiguousarray(wo2, dtype=ml_dtypes.bfloat16),
            "onesb_d": np.ones((128, 8), ml_dtypes.bfloat16),
            "identb_d": np.eye(128, dtype=ml_dtypes.bfloat16),
        })
    return in_maps


def kernel(x, W_qkv, b_qkv, W_out, b_out):
    in_maps = prepare_in_maps(x, W_qkv, b_qkv, W_out, b_out)
    res = run_bass_kernel_spmd(get_nc(), in_maps, core_ids=list(range(NCORES)))
    acc = np.zeros((B * L, C), np.float64)
    for core_out in res.results:
        acc += core_out["out"]
    acc += np.asarray(b_out, np.float64)[None, :]
    return acc.reshape(B, L, C).astype(np.float32)


if __name__ == "__main__":
    rng = np.random.default_rng(0)
    ins = {
        "x": rng.standard_normal((B, L, C)).astype(np.float32),
        "W_qkv": rng.uniform(-1 / 32, 1 / 32, (3 * C, C)).astype(np.float32),
        "b_qkv": rng.uniform(-1 / 32, 1 / 32, (3 * C,)).astype(np.float32),
        "W_out": rng.uniform(-1 / 32, 1 / 32, (C, C)).astype(np.float32),
        "b_out": rng.uniform(-1 / 32, 1 / 32, (C,)).astype(np.float32),
    }
    o = kernel(**ins)
    print(o.shape, o.dtype)


# revision 17
# speedup vs baseline: 1.4100x; 1.4100x over previous
"""Multi-head attention (B=4, L=2048, C=1024, H=16, HD=64) on 8 NeuronCores.

Sharding: tensor-parallel over heads — 2 heads per core. Each core computes
its heads' QKV projection, attention, and a partial output projection over
its 128 ctx channels; the host sums the 8 partial outputs.

Per-core kernel layout notes:
  - All projections/attention keep "T" layouts (channels on partitions);
    the whole data path is bf16 except PSUM accumulation (fp32) and the
    softmax normalization chain.
  - Scores matmuls are K=64 per head, row-tiled onto the PE array with
    tile_position (0,0)/(64,0) so both heads' scores run CONCURRENTLY in
    disjoint row halves of the systolic array (q/k tiles carry head0 dims
    on partitions 0:64 and head1 on 64:128, matching the array row groups).
  - ctx matmuls are M=64 per head, col-tiled with tile_position (0,0)/(0,64)
    so both heads' ctx accumulate CONCURRENTLY into partition halves of one
    PSUM bank.
  - Softmax skips the max-subtraction (scores are ~N(0, 1/9): exp is safe).
    The denominator Z comes from a bf16 pairwise tree-reduction of the 16
    exp tiles (DVE) + one M=1 ones-matmul per head; 1/Z via DVE
    reciprocal_approx_fast (no ACT Exp<->Ln table reloads).
  - DMA descriptors are merged (one DMA for all qkv weights, one per
    512-token x chunk, one per outproj row tile) to keep the Sync queue
    short — descriptor issue is ~0.6us each.
"""

import numpy as np
import ml_dtypes

import concourse.bass as bass
import concourse.mybir as mybir
import concourse.tile as tile
from concourse import bacc
from concourse.bass_utils import run_bass_kernel_spmd

B, L, C, H, HD = 4, 2048, 1024, 16, 64
NCORES = 8
HPC = H // NCORES  # heads per core = 2
F32 = mybir.dt.float32
F32R = mybir.dt.float32r
BF16 = mybir.dt.bfloat16
EXP = mybir.ActivationFunctionType.Exp

LCHUNK = 512          # token chunk for moving operands
NLC = L // LCHUNK     # 4
NKT = L // 128        # 16 k tiles per sequence
NCT = C // 128        # 8 contraction tiles for the projections


def build_kernel():
    nc = bacc.Bacc("TRN2", target_bir_lowering=False, debug=False,
                   num_devices=NCORES)

    xT = nc.dram_tensor("xT", [B, C, L], BF16, kind="ExternalInput")
    # wqkv[ci, j] = [128 c, 128 f] tile; j in (0=q both heads, 1=k, 2=v)
    wqkv = nc.dram_tensor("wqkv", [NCT, 3, 128, 128], BF16, kind="ExternalInput")
    bqkv = nc.dram_tensor("bqkv", [3, 128, 1], F32, kind="ExternalInput")
    # wo2: [128 c(2 heads), 1024 o]
    wo2 = nc.dram_tensor("wo2", [128, C], BF16, kind="ExternalInput")
    onesb_d = nc.dram_tensor("onesb_d", [128, 8], BF16, kind="ExternalInput")
    identb_d = nc.dram_tensor("identb_d", [128, 128], BF16, kind="ExternalInput")
    out = nc.dram_tensor("out", [B * L, C], F32, kind="ExternalOutput")

    with tile.TileContext(nc) as tc:
        kernel_body(nc, tc, xT, wqkv, bqkv, wo2, onesb_d, identb_d, out)
    nc.compile()
    return nc


def kernel_body(nc, tc, xT, wqkv, bqkv, wo2, onesb_d, identb_d, out):
    from contextlib import ExitStack
    ctx = ExitStack()
    with ctx:
        consts = ctx.enter_context(tc.tile_pool(name="consts", bufs=1))
        xpool = ctx.enter_context(tc.tile_pool(name="xpool", bufs=4))
        qkvpool = ctx.enter_context(tc.tile_pool(name="qkvpool", bufs=2))
        vppool = ctx.enter_context(tc.tile_pool(name="vppool", bufs=24))
        epool = ctx.enter_context(tc.tile_pool(name="epool", bufs=18))
        cpool = ctx.enter_context(tc.tile_pool(name="cpool", bufs=2))
        spool = ctx.enter_context(tc.tile_pool(name="spool", bufs=2))
        opool = ctx.enter_context(tc.tile_pool(name="opool", bufs=3))
        # PSUM banks: s pairs 2x2 + cacc 2 + general 2 = 8
        spsum = ctx.enter_context(tc.tile_pool(name="spsum", bufs=2,
                                               space="PSUM"))
        cpsum = ctx.enter_context(tc.tile_pool(name="cpsum", bufs=2,
                                               space="PSUM"))
        gpsum = ctx.enter_context(tc.tile_pool(name="gpsum", bufs=2,
                                               space="PSUM"))

        # ---- constants (single merged weight DMA) ----
        w_all = consts.tile([128, NCT, 3, 128], BF16, tag="w_all")
        nc.sync.dma_start(out=w_all, in_=wqkv[:].rearrange("a b c d -> c a b d"))
        w_tiles = [[w_all[:, ci, j, :] for j in range(3)] for ci in range(NCT)]
        b_tiles = []
        for j in range(3):
            t = consts.tile([128, 1], F32, tag=f"b{j}")
            nc.sync.dma_start(out=t, in_=bqkv[j])
            b_tiles.append(t)
        wo_t = consts.tile([128, C], BF16, tag="wo_t")
        nc.sync.dma_start(out=wo_t, in_=wo2[:])
        onesb = consts.tile([128, 8], BF16, tag="onesb")
        nc.sync.dma_start(out=onesb, in_=onesb_d[:])
        identb = consts.tile([128, 128], BF16, tag="identb")
        nc.sync.dma_start(out=identb, in_=identb_d[:])

        # ---- phase helpers (emitted in software-pipelined order below) ----
        def emit_qkv_loads(b, lc):
            ls = bass.ts(lc, LCHUNK)
            xt = xpool.tile([128, NCT, LCHUNK], BF16, tag="xt", name="xt")
            nc.sync.dma_start(
                out=xt, in_=xT[b, :, ls].rearrange("(a p) l -> p a l", p=128))
            return xt

        def emit_qkv_block(b, lc, qkvT, xt):
            ls = bass.ts(lc, LCHUNK)
            for j in range(3):
                p = gpsum.tile([128, LCHUNK], F32, tag="gpb", name="p")
                for ci in range(NCT):
                    nc.tensor.matmul(p, w_tiles[ci][j], xt[:, ci, :],
                                     start=(ci == 0), stop=(ci == NCT - 1))
                # PSUM -> SBUF with per-partition bias add
                nc.vector.tensor_scalar_add(qkvT[j][:, ls], p, b_tiles[j][:])

        def emit_vplus(qkvT):
            # v -> token-major bf16 tiles [128 l][2 h][v_h | 1]
            vplus = []
            for t in range(NKT):
                tp = gpsum.tile([128, 128], BF16, tag="gpb", name="tp")
                nc.tensor.transpose(tp, qkvT[2][:, bass.ts(t, 128)], identb[:])
                vp = vppool.tile([128, 2, HD + 1], BF16, tag="vp", name="vp")
                nc.vector.tensor_copy(
                    vp[:, :, 0:HD],
                    tp[:, :].rearrange("p (a b) -> p a b", a=2))
                nc.vector.tensor_copy(
                    vp[:, :, HD:HD + 1],
                    onesb[:, 0:2].rearrange("p (a b) -> p a b", a=2))
                vplus.append(vp)
            return vplus

        def emit_attn_chunk(qc, qkvT, vplus):
            # both heads, one 512-wide q chunk; scores run as K=64 row-tiled
            # pairs (head h in array rows 64h:64h+64), ctx as M=64 col-tiled
            # pairs (head h in array cols 64h:64h+64); one exp per k-tile
            # covering both heads' PSUM banks.
            qs = bass.ts(qc, LCHUNK)
            qt, kt = qkvT[0], qkvT[1]
            evec = []
            for i in range(NKT):
                ks = bass.ts(i, 128)
                s = spsum.tile([128, 2 * LCHUNK], F32, tag="spb", name="s")
                nc.tensor.matmul(s[:, 0:LCHUNK], kt[0:HD, ks], qt[0:HD, qs],
                                 start=True, stop=True, tile_position=(0, 0))
                nc.tensor.matmul(s[:, LCHUNK:2 * LCHUNK], kt[HD:128, ks],
                                 qt[HD:128, qs],
                                 start=True, stop=True, tile_position=(64, 0))
                e = epool.tile([128, 2 * LCHUNK], BF16, tag="e", name="e")
                nc.scalar.activation(e, s, EXP, scale=0.125)
                evec.append(e)
            # ctx accumulation per head, M=65 (64 dims + ones column whose
            # output row IS the softmax denominator Z)
            caccs = [cpsum.tile([HD + 1, LCHUNK], F32, tag="cpb",
                                name=f"cacc{h}") for h in range(HPC)]
            for i in range(NKT):
                for h in range(HPC):
                    nc.tensor.matmul(
                        caccs[h],
                        vplus[i][:, h, :],
                        evec[i][:, bass.ts(h, LCHUNK)],
                        start=(i == 0), stop=(i == NKT - 1))
            # drain cacc immediately (frees the PSUM banks; rowsum row
            # lands on partition 64)
            pend = []
            for h in range(HPC):
                cacc = caccs[h]
                csb = spool.tile([HD, LCHUNK], F32, tag="csb", name="csb",
                                 bufs=5)
                nc.vector.tensor_copy(csb, cacc[0:HD, :])
                z0 = spool.tile([1, LCHUNK], F32, tag="z0", name="z0", bufs=5)
                nc.vector.tensor_copy(z0[0:1, :], cacc[HD:HD + 1, :])
                pend.append((h, qs, csb, z0))
            return pend

        def emit_normalize(pend, ctxT2):
            # broadcast Z from partition 0 (GpSimd broadcast only reads
            # physical partition 0), then one full-width DVE reciprocal
            # and multiply per head
            for (h, qs, csb, z0) in pend:
                zb = spool.tile([HD, LCHUNK], F32, tag="zb", name="zb", bufs=3)
                nc.gpsimd.partition_broadcast(zb[0:HD, :], z0[0:1, :])
                zs = spool.tile([HD, LCHUNK], F32, tag="zs", name="zs", bufs=3)
                nc.vector.reciprocal_approx_fast(zs, zb)
                nc.vector.tensor_mul(ctxT2[h * HD:h * HD + HD, qs],
                                     csb[0:HD, :], zs)

        def emit_outproj(b, ctxT2, trange):
            for t in trange:
                rows = bass.ds(b * L + t * 128, 128)
                ot = opool.tile([128, C], F32, tag="ot", name="ot")
                for oc in range(C // 512):
                    os_ = bass.ts(oc, 512)
                    o = gpsum.tile([128, 512], F32, tag="gpb", name="o")
                    nc.tensor.matmul(o, ctxT2[:, bass.ts(t, 128)],
                                     wo_t[:, os_], start=True, stop=True)
                    nc.vector.tensor_copy(ot[:, os_], o)
                nc.sync.dma_start(out=out[rows, :], in_=ot)

        # ---- software-pipelined emission ----
        # Interleave next batch's qkv blocks between attention chunks so the
        # PE's in-order queue always has dense, ready work behind any stall.
        def new_qkvT():
            qp = qkvpool.tile([128, L], BF16, tag="qp", name="qp")
            kc = qkvpool.tile([128, L], BF16, tag="kc", name="kc")
            vc = qkvpool.tile([128, L], BF16, tag="vc", name="vc")
            return [qp, kc, vc]

        qkvT = new_qkvT()
        for lc in range(NLC):
            xt = emit_qkv_loads(0, lc)
            emit_qkv_block(0, lc, qkvT, xt)
        vplus = emit_vplus(qkvT)
        for b in range(B):
            ctxT2 = cpool.tile([128, L], BF16, tag="ctxT2", name="ctxT2")
            nxt = new_qkvT() if b + 1 < B else None
            loads = {}
            if nxt is not None:
                loads[0] = emit_qkv_loads(b + 1, 0)
                loads[1] = emit_qkv_loads(b + 1, 1)
            for k in range(NLC):
                pend = emit_attn_chunk(k, qkvT, vplus)
                if nxt is not None:
                    if k + 2 < NLC:
                        loads[k + 2] = emit_qkv_loads(b + 1, k + 2)
                    emit_qkv_block(b + 1, k, nxt, loads.pop(k))
                emit_normalize(pend, ctxT2)
                if k == 1:
                    emit_outproj(b, ctxT2, range(0, NKT // 2))
            if nxt is not None:
                nxt_vplus = emit_vplus(nxt)
            emit_outproj(b, ctxT2, range(NKT // 2, NKT))
            if nxt is not None:
                qkvT, vplus = nxt, nxt_vplus


_NC_CACHE = None


def get_nc():
    global _NC_CACHE
    if _NC_CACHE is None:
        _NC_CACHE = build_kernel()
    return _NC_CACHE


def prepare_in_maps(x, W_qkv, b_qkv, W_out, b_out):
    x = np.asarray(x, np.float32)
    W_qkv = np.asarray(W_qkv, np.float32)
    b_qkv = np.asarray(b_qkv, np.float32)
    W_out = np.asarray(W_out, np.float32)
    b_out = np.asarray(b_out, np.float32)

    xT = np.ascontiguousarray(x.transpose(0, 2, 1))  # [B, C, L]

    in_maps = []
    for core in range(NCORES):
        h0 = HPC * core
        # per-head channel rows in W_qkv: q = h*192..+64, k = +64, v = +128
        qrows = [np.arange(h * 192, h * 192 + 64) for h in (h0, h0 + 1)]
        krows = [q + 64 for q in qrows]
        vrows = [q + 128 for q in qrows]
        fq = np.concatenate(qrows)
        fk = np.concatenate(krows)
        fv = np.concatenate(vrows)
        # wqkv tiles: [ci, j, 128 c, 128 f]
        wt = np.empty((NCT, 3, 128, 128), ml_dtypes.bfloat16)
        for j, rows in enumerate((fq, fk, fv)):
            wT = np.ascontiguousarray(W_qkv[rows].T)  # [1024 c, 128 f]
            wt[:, j] = wT.reshape(NCT, 128, 128)
        bq = np.stack([b_qkv[fq], b_qkv[fk], b_qkv[fv]])[..., None]  # [3,128,1]
        # wo2 = [128 c, 1024 o]: rows 0:64 h0 ctx channels, 64:128 h1
        wo2 = np.concatenate([
            np.ascontiguousarray(W_out[:, (h0 + h) * HD:(h0 + h + 1) * HD].T)
            for h in range(HPC)
        ], axis=0)
        in_maps.append({
            "xT": xT.astype(ml_dtypes.bfloat16),
            "wqkv": wt,
            "bqkv": np.ascontiguousarray(bq),
            "wo2": np.ascontiguousarray(wo2, dtype=ml_dtypes.bfloat16),
            "onesb_d": np.ones((128, 8), ml_dtypes.bfloat16),
            "identb_d": np.eye(128, dtype=ml_dtypes.bfloat16),
        })
    return in_maps


def kernel(x, W_qkv, b_qkv, W_out, b_out):
    in_maps = prepare_in_maps(x, W_qkv, b_qkv, W_out, b_out)
    res = run_bass_kernel_spmd(get_nc(), in_maps, core_ids=list(range(NCORES)))
    acc = np.zeros((B * L, C), np.float64)
    for core_out in res.results:
        acc += core_out["out"]
    acc += np.asarray(b_out, np.float64)[None, :]
    return acc.reshape(B, L, C).astype(np.float32)


if __name__ == "__main__":
    rng = np.random.default_rng(0)
    ins = {
        "x": rng.standard_normal((B, L, C)).astype(np.float32),
        "W_qkv": rng.uniform(-1 / 32, 1 / 32, (3 * C, C)).astype(np.float32),
        "b_qkv": rng.uniform(-1 / 32, 1 / 32, (3 * C,)).astype(np.float32),
        "W_out": rng.uniform(-1 / 32, 1 / 32, (C, C)).astype(np.float32),
        "b_out": rng.uniform(-1 / 32, 1 / 32, (C,)).astype(np.float32),
    }
    o = kernel(**ins)
    print(o.shape, o.dtype)


# revision 24
# speedup vs baseline: 1.4511x; 1.0291x over previous
"""Multi-head attention (B=4, L=2048, C=1024, H=16, HD=64) on 8 NeuronCores.

Sharding: tensor-parallel over heads — 2 heads per core. Each core computes
its heads' QKV projection, attention, and a partial output projection over
its 128 ctx channels; the host sums the 8 partial outputs.

Per-core kernel layout notes:
  - All projections/attention keep "T" layouts (channels on partitions);
    the whole data path is bf16 except PSUM accumulation (fp32) and the
    softmax normalization chain. bf16 keeps LDWEIGHTS at ~107ns (fp32r
    4-byte weight loads measured ~224ns and gate the projection phases).
  - Scores matmuls are K=64 per head, row-tiled onto the PE array with
    tile_position (0,0)/(64,0) so both heads' scores run CONCURRENTLY in
    disjoint row halves of the systolic array (q/k tiles carry head0 dims
    on partitions 0:64 and head1 on 64:128, matching the array row groups;
    bf16 moving streams share the input bus — fp32r pairs serialize).
  - ctx matmuls are M=65 per head: 64 v-dims plus a ones column whose
    output row IS the softmax denominator Z (a separate Z reduction is
    not viable: DVE tensor_tensor is ~2.6us per [128,1024] tile and
    GpSimd contends with DVE for the shared SBUF port).
  - Softmax skips the max-subtraction (scores are ~N(0, 1/9): exp is
    safe); 1/Z via GpSimd partition-0 broadcast + DVE
    reciprocal_approx_fast (no ACT Exp<->Ln table reloads).
  - Emission is software-pipelined one dense PE stream: chunk k's score
    pairs interleave chunk k-1's ctx matmuls plus queued "extras" (next
    batch's qkv j-groups, v transposes, finished chunks' outproj tiles),
    so the ACT exp stream (256 x 1.1us, the second-busiest engine) never
    starves at chunk boundaries and HAM stays at full clock.
  - DMA descriptors are merged (one DMA for all qkv weights, one per
    512-token x chunk, one per outproj row tile) to keep the Sync queue
    short — descriptor issue is ~0.6us each.
"""

import numpy as np
import ml_dtypes

import concourse.bass as bass
import concourse.mybir as mybir
import concourse.tile as tile
from concourse import bacc
from concourse.bass_utils import run_bass_kernel_spmd

B, L, C, H, HD = 4, 2048, 1024, 16, 64
NCORES = 8
HPC = H // NCORES  # heads per core = 2
F32 = mybir.dt.float32
F32R = mybir.dt.float32r
BF16 = mybir.dt.bfloat16
EXP = mybir.ActivationFunctionType.Exp

LCHUNK = 512          # token chunk for moving operands
NLC = L // LCHUNK     # 4
NKT = L // 128        # 16 k tiles per sequence
NCT = C // 128        # 8 contraction tiles for the projections


def build_kernel():
    nc = bacc.Bacc("TRN2", target_bir_lowering=False, debug=False,
                   num_devices=NCORES)

    xT = nc.dram_tensor("xT", [B, C, L], BF16, kind="ExternalInput")
    # wqkv[ci, j] = [128 c, 128 f] tile; j in (0=q both heads, 1=k, 2=v)
    wqkv = nc.dram_tensor("wqkv", [NCT, 3, 128, 128], BF16, kind="ExternalInput")
    bqkv = nc.dram_tensor("bqkv", [3, 128, 1], F32, kind="ExternalInput")
    # wo2: [128 c(2 heads), 1024 o]
    wo2 = nc.dram_tensor("wo2", [128, C], BF16, kind="ExternalInput")
    onesb_d = nc.dram_tensor("onesb_d", [128, 8], BF16, kind="ExternalInput")
    identb_d = nc.dram_tensor("identb_d", [128, 128], BF16, kind="ExternalInput")
    out = nc.dram_tensor("out", [B * L, C], F32, kind="ExternalOutput")

    with tile.TileContext(nc) as tc:
        kernel_body(nc, tc, xT, wqkv, bqkv, wo2, onesb_d, identb_d, out)
    nc.compile()
    return nc


def kernel_body(nc, tc, xT, wqkv, bqkv, wo2, onesb_d, identb_d, out):
    from contextlib import ExitStack
    ctx = ExitStack()
    with ctx:
        consts = ctx.enter_context(tc.tile_pool(name="consts", bufs=1))
        xpool = ctx.enter_context(tc.tile_pool(name="xpool", bufs=4))
        qkvpool = ctx.enter_context(tc.tile_pool(name="qkvpool", bufs=2))
        vppool = ctx.enter_context(tc.tile_pool(name="vppool", bufs=24))
        epool = ctx.enter_context(tc.tile_pool(name="epool", bufs=18))
        cpool = ctx.enter_context(tc.tile_pool(name="cpool", bufs=2))
        spool = ctx.enter_context(tc.tile_pool(name="spool", bufs=2))
        opool = ctx.enter_context(tc.tile_pool(name="opool", bufs=3))
        # PSUM banks: s pairs 2x2 + cacc 2 + general 2 = 8
        spsum = ctx.enter_context(tc.tile_pool(name="spsum", bufs=2,
                                               space="PSUM"))
        cpsum = ctx.enter_context(tc.tile_pool(name="cpsum", bufs=2,
                                               space="PSUM"))
        gpsum = ctx.enter_context(tc.tile_pool(name="gpsum", bufs=2,
                                               space="PSUM"))

        # ---- batch-0 x loads first (they gate the first matmul) ----
        xt0 = []

        def emit_qkv_loads(b, lc):
            ls = bass.ts(lc, LCHUNK)
            xt = xpool.tile([128, NCT, LCHUNK], BF16, tag="xt", name="xt")
            nc.sync.dma_start(
                out=xt, in_=xT[b, :, ls].rearrange("(a p) l -> p a l", p=128))
            return xt

        xt0.append(emit_qkv_loads(0, 0))

        # ---- constants (single merged weight DMA) ----
        w_all = consts.tile([128, NCT, 3, 128], BF16, tag="w_all")
        nc.sync.dma_start(out=w_all, in_=wqkv[:].rearrange("a b c d -> c a b d"))
        w_tiles = [[w_all[:, ci, j, :] for j in range(3)] for ci in range(NCT)]
        b_tiles = []
        for j in range(3):
            t = consts.tile([128, 1], F32, tag=f"b{j}")
            nc.sync.dma_start(out=t, in_=bqkv[j])
            b_tiles.append(t)
        wo_t = consts.tile([128, C], BF16, tag="wo_t")
        nc.sync.dma_start(out=wo_t, in_=wo2[:])
        onesb = consts.tile([128, 8], BF16, tag="onesb")
        nc.sync.dma_start(out=onesb, in_=onesb_d[:])
        identb = consts.tile([128, 128], BF16, tag="identb")
        nc.sync.dma_start(out=identb, in_=identb_d[:])

        # ---- phase helpers (emitted in software-pipelined order below) ----
        def emit_qkv_block(b, lc, qkvT, xt):
            ls = bass.ts(lc, LCHUNK)
            for j in range(3):
                p = gpsum.tile([128, LCHUNK], F32, tag="gpb", name="p")
                for ci in range(NCT):
                    nc.tensor.matmul(p, w_tiles[ci][j], xt[:, ci, :],
                                     start=(ci == 0), stop=(ci == NCT - 1))
                # PSUM -> SBUF with per-partition bias add
                nc.vector.tensor_scalar_add(qkvT[j][:, ls], p, b_tiles[j][:])

        def emit_vplus(qkvT):
            # v -> token-major bf16 tiles [128 l][2 h][v_h | 1]
            vplus = []
            for t in range(NKT):
                tp = gpsum.tile([128, 128], BF16, tag="gpb", name="tp")
                nc.tensor.transpose(tp, qkvT[2][:, bass.ts(t, 128)], identb[:])
                vp = vppool.tile([128, 2, HD + 1], BF16, tag="vp", name="vp")
                nc.vector.tensor_copy(
                    vp[:, :, 0:HD],
                    tp[:, :].rearrange("p (a b) -> p a b", a=2))
                nc.vector.tensor_copy(
                    vp[:, :, HD:HD + 1],
                    onesb[:, 0:2].rearrange("p (a b) -> p a b", a=2))
                vplus.append(vp)
            return vplus

        def emit_scores(qkvT, qc, filler):
            # 16 k-tiles of row-tiled score pairs + exp; filler(i) emits
            # other PE work between pairs (prev chunk's ctx, qkv j-groups,
            # v transposes) so the PE stream stays dense while the exp
            # stream stays fed
            qs = bass.ts(qc, LCHUNK)
            qt, kt = qkvT[0], qkvT[1]
            evec = []
            for i in range(NKT):
                ks = bass.ts(i, 128)
                s = spsum.tile([128, 2 * LCHUNK], F32, tag="spb", name="s")
                nc.tensor.matmul(s[:, 0:LCHUNK], kt[0:HD, ks], qt[0:HD, qs],
                                 start=True, stop=True, tile_position=(0, 0))
                nc.tensor.matmul(s[:, LCHUNK:2 * LCHUNK], kt[HD:128, ks],
                                 qt[HD:128, qs],
                                 start=True, stop=True, tile_position=(64, 0))
                e = epool.tile([128, 2 * LCHUNK], BF16, tag="e", name="e")
                nc.scalar.activation(e, s, EXP, scale=0.125)
                evec.append(e)
                filler(i)
            return evec

        class Chunk:
            pass

        extras_q = []

        def ctx_pair(st, i):
            # ctx accumulation per head, M=65 (64 dims + ones column whose
            # output row IS the softmax denominator Z)
            for h in range(HPC):
                nc.tensor.matmul(
                    st.caccs[h],
                    st.vplus[i][:, h, :],
                    st.evec[i][:, bass.ts(h, LCHUNK)],
                    start=(i == 0), stop=(i == NKT - 1))

        def finish_chunk(st):
            # drain cacc (frees the PSUM banks; rowsum Z lands on partition
            # 64), broadcast Z from partition 0 (GpSimd broadcast only reads
            # physical partition 0), 1/Z on DVE, normalize into ctxT2
            for h in range(HPC):
                cacc = st.caccs[h]
                csb = spool.tile([HD, LCHUNK], F32, tag="csb", name="csb",
                                 bufs=4)
                nc.vector.tensor_copy(csb, cacc[0:HD, :])
                z0 = spool.tile([1, LCHUNK], F32, tag="z0", name="z0", bufs=4)
                nc.vector.tensor_copy(z0[0:1, :], cacc[HD:HD + 1, :])
                zb = spool.tile([HD, LCHUNK], F32, tag="zb", name="zb", bufs=2)
                nc.gpsimd.partition_broadcast(zb[0:HD, :], z0[0:1, :])
                zs = spool.tile([HD, LCHUNK], F32, tag="zs", name="zs", bufs=2)
                nc.vector.reciprocal_approx_fast(zs, zb)
                nc.vector.tensor_mul(st.ctxT2[h * HD:h * HD + HD, st.qs],
                                     csb[0:HD, :], zs)
            # this chunk's tokens are now normalized -> queue their
            # outproj tiles as score-loop fillers
            for t in range(4 * st.k, 4 * st.k + 4):
                def go(b=st.b, ctxT2=st.ctxT2, t=t):
                    emit_outproj(b, ctxT2, [t])
                extras_q.append(go)

        def emit_outproj(b, ctxT2, trange):
            for t in trange:
                rows = bass.ds(b * L + t * 128, 128)
                ot = opool.tile([128, C], F32, tag="ot", name="ot", bufs=2)
                for oc in range(C // 512):
                    os_ = bass.ts(oc, 512)
                    o = gpsum.tile([128, 512], F32, tag="gpb", name="o")
                    nc.tensor.matmul(o, ctxT2[:, bass.ts(t, 128)],
                                     wo_t[:, os_], start=True, stop=True)
                    nc.vector.tensor_copy(ot[:, os_], o)
                nc.sync.dma_start(out=out[rows, :], in_=ot)

        def qkv_extras(b, qkvT):
            # per-j-group callables for batch b's qkv projection; loads are
            # issued immediately, matmuls deferred into score-loop fillers
            ext = []
            for lc in range(NLC):
                xt = emit_qkv_loads(b, lc)
                for j in range(3):
                    def go(lc=lc, j=j, xt=xt):
                        ls = bass.ts(lc, LCHUNK)
                        p = gpsum.tile([128, LCHUNK], F32, tag="gpb", name="p")
                        for ci in range(NCT):
                            nc.tensor.matmul(p, w_tiles[ci][j], xt[:, ci, :],
                                             start=(ci == 0),
                                             stop=(ci == NCT - 1))
                        nc.vector.tensor_scalar_add(qkvT[j][:, ls], p,
                                                    b_tiles[j][:])
                    ext.append(go)
            return ext

        def vplus_extras(qkvT, vplus):
            # per-k-tile callables: v -> token-major [128 l][2 h][v_h | 1]
            ext = []
            for t in range(NKT):
                def go(t=t):
                    tp = gpsum.tile([128, 128], BF16, tag="gpb", name="tp")
                    nc.tensor.transpose(tp, qkvT[2][:, bass.ts(t, 128)],
                                        identb[:])
                    vp = vppool.tile([128, 2, HD + 1], BF16, tag="vp",
                                     name="vp")
                    nc.vector.tensor_copy(
                        vp[:, :, 0:HD],
                        tp[:, :].rearrange("p (a b) -> p a b", a=2))
                    nc.vector.tensor_copy(
                        vp[:, :, HD:HD + 1],
                        onesb[:, 0:2].rearrange("p (a b) -> p a b", a=2))
                    vplus[t] = vp
                ext.append(go)
            return ext

        # ---- software-pipelined emission ----
        # Chunk k's score pairs interleave chunk k-1's ctx matmuls plus the
        # next batch's qkv j-groups and v transposes, so the PE queue is one
        # dense stream and the ACT exp stream never starves at chunk breaks.
        def new_qkvT():
            qp = qkvpool.tile([128, L], BF16, tag="qp", name="qp")
            kc = qkvpool.tile([128, L], BF16, tag="kc", name="kc")
            vc = qkvpool.tile([128, L], BF16, tag="vc", name="vc")
            return [qp, kc, vc]

        def make_filler(prev):
            def fill(i):
                if prev is not None:
                    ctx_pair(prev, i)
                if extras_q:
                    extras_q.pop(0)()
            return fill

        qkvT = new_qkvT()
        for lc in range(NLC):
            xt = xt0[0] if lc == 0 else emit_qkv_loads(0, lc)
            emit_qkv_block(0, lc, qkvT, xt)
        vplus = [None] * NKT
        extras_q += vplus_extras(qkvT, vplus)
        prev = None
        for b in range(B):
            ctxT2 = cpool.tile([128, L], BF16, tag="ctxT2", name="ctxT2")
            nxt = new_qkvT() if b + 1 < B else None
            nxt_vplus = [None] * NKT
            if nxt is not None:
                extras_q.extend(qkv_extras(b + 1, nxt))
                extras_q.extend(vplus_extras(nxt, nxt_vplus))
            for k in range(NLC):
                st = Chunk()
                st.b, st.k, st.qs = b, k, bass.ts(k, LCHUNK)
                st.vplus, st.ctxT2 = list(vplus), ctxT2
                st.evec = emit_scores(qkvT, k, make_filler(prev))
                if prev is not None:
                    finish_chunk(prev)
                st.caccs = [cpsum.tile([HD + 1, LCHUNK], F32, tag="cpb",
                                       name=f"cacc{h}") for h in range(HPC)]
                prev = st
            if nxt is not None:
                qkvT, vplus = nxt, nxt_vplus
        # tail: last chunk's ctx, then drain already-ready extras (the
        # second-to-last chunk's outproj tiles — they don't depend on the
        # last normalize) to keep the PE busy under the final DVE chain
        for i in range(NKT):
            ctx_pair(prev, i)
            if extras_q:
                extras_q.pop(0)()
        while extras_q:
            extras_q.pop(0)()
        finish_chunk(prev)
        while extras_q:
            extras_q.pop(0)()


_NC_CACHE = None


def get_nc():
    global _NC_CACHE
    if _NC_CACHE is None:
        _NC_CACHE = build_kernel()
    return _NC_CACHE


def prepare_in_maps(x, W_qkv, b_qkv, W_out, b_out):
    x = np.asarray(x, np.float32)
    W_qkv = np.asarray(W_qkv, np.float32)
    b_qkv = np.asarray(b_qkv, np.float32)
    W_out = np.asarray(W_out, np.float32)
    b_out = np.asarray(b_out, np.float32)

    xT = np.ascontiguousarray(x.transpose(0, 2, 1))  # [B, C, L]

    in_maps = []
    for core in range(NCORES):
        h0 = HPC * core
        # per-head channel rows in W_qkv: q = h*192..+64, k = +64, v = +128
        qrows = [np.arange(h * 192, h * 192 + 64) for h in (h0, h0 + 1)]
        krows = [q + 64 for q in qrows]
        vrows = [q + 128 for q in qrows]
        fq = np.concatenate(qrows)
        fk = np.concatenate(krows)
        fv = np.concatenate(vrows)
        # wqkv tiles: [ci, j, 128 c, 128 f]
        wt = np.empty((NCT, 3, 128, 128), ml_dtypes.bfloat16)
        for j, rows in enumerate((fq, fk, fv)):
            wT = np.ascontiguousarray(W_qkv[rows].T)  # [1024 c, 128 f]
            wt[:, j] = wT.reshape(NCT, 128, 128)
        bq = np.stack([b_qkv[fq], b_qkv[fk], b_qkv[fv]])[..., None]  # [3,128,1]
        # wo2 = [128 c, 1024 o]: rows 0:64 h0 ctx channels, 64:128 h1
        wo2 = np.concatenate([
            np.ascontiguousarray(W_out[:, (h0 + h) * HD:(h0 + h + 1) * HD].T)
            for h in range(HPC)
        ], axis=0)
        in_maps.append({
            "xT": xT.astype(ml_dtypes.bfloat16),
            "wqkv": wt,
            "bqkv": np.ascontiguousarray(bq),
            "wo2": np.ascontiguousarray(wo2, dtype=ml_dtypes.bfloat16),
            "onesb_d": np.ones((128, 8), ml_dtypes.bfloat16),
            "identb_d": np.eye(128, dtype=ml_dtypes.bfloat16),
        })
    return in_maps


def kernel(x, W_qkv, b_qkv, W_out, b_out):
    in_maps = prepare_in_maps(x, W_qkv, b_qkv, W_out, b_out)
    res = run_bass_kernel_spmd(get_nc(), in_maps, core_ids=list(range(NCORES)))
    acc = np.zeros((B * L, C), np.float64)
    for core_out in res.results:
        acc += core_out["out"]
    acc += np.asarray(b_out, np.float64)[None, :]
    return acc.reshape(B, L, C).astype(np.float32)


if __name__ == "__main__":
    rng = np.random.default_rng(0)
    ins = {
        "x": rng.standard_normal((B, L, C)).astype(np.float32),
        "W_qkv": rng.uniform(-1 / 32, 1 / 32, (3 * C, C)).astype(np.float32),
        "b_qkv": rng.uniform(-1 / 32, 1 / 32, (3 * C,)).astype(np.float32),
        "W_out": rng.uniform(-1 / 32, 1 / 32, (C, C)).astype(np.float32),
        "b_out": rng.uniform(-1 / 32, 1 / 32, (C,)).astype(np.float32),
    }
    o = kernel(**ins)
    print(o.shape, o.dtype)
